# revision 12
# baseline (speedup 1.0000x reference)
"""Trainium2 Bass kernel for nn_ContextAggregation (channel attention).

Reference computation (per batch b of 4, c=384, nh=8, hd=48, hw=128*128):
    qkv  = qkv_w @ x            # [1152, hw] channel GEMM
    q,k,v split; q,k L2-normalized along hw
    attn = softmax((q_hat @ k_hat^T) * tau)   # [nh, 48, 48]
    out  = proj_w @ (attn @ v) + proj_b

Sharding: 8 cores = (batch b, hw-half h).  Each core processes hw half
(8192 px) of one batch.  The only cross-core data needed are the gram
matrices q@k^T and the row sums-of-squares (for the L2 norms), both of
which are sums over hw -> one tiny pair AllReduce ([98,384] floats)
between the two cores sharing a batch.  All heavy GEMMs are local.

Per-core pipeline (single Bass program, SPMD on cores 0-7):
  phase 1 (stream 16 chunks of 512 px):
    - v    = Wv @ x                        (natural [ch, hw] layout, fp32r)
    - qT,kT= x^T @ Wq^T / Wk^T             (transposed [hw, ch] layout: the
             x subtile is the stationary operand, so q/k come out hw-major
             and the gram contraction over hw can use the PE directly)
    - G   += qT_pair^T @ kT_pair           (4 head-pairs, PSUM-accumulated)
    - ssq += ones^T @ qT^2, ssk            (PSUM-accumulated sumsq)
  AllReduce(G|ssq|ssk) within batch pairs.
  phase 2/3: inverse norms, softmax per head, transpose S via PE,
             scatter into a block-diagonal S^T (384x384, fp32r).
  phase 4 (stream 16 chunks):
    - out  = S_bd^T @ v                    (skips all-zero block tiles)
    - y    = projT^T @ out + b             -> DMA to DRAM
"""

import numpy as np

import concourse.bass as bass
import concourse.mybir as mybir
import concourse.tile as tile
from concourse import bacc
from concourse.bass_utils import run_bass_kernel_spmd
from concourse.bass_interp import get_hw_module
from concourse.masks import make_identity

dt = mybir.dt
F32 = dt.float32
F32R = dt.float32r
ALU = mybir.AluOpType
AFT = mybir.ActivationFunctionType
AX = mybir.AxisListType

C = 384          # channels
NH = 8           # heads
HD = 48          # head dim
HW = 128 * 128   # pixels per image
HALF = HW // 2   # pixels per core
CHUNK = 512
NCHUNK = HALF // CHUNK          # 16
SUB = 128
NSUB_PER_CHUNK = CHUNK // SUB   # 4
N_CORES = 8
GROUPS = [[0, 1], [2, 3], [4, 5], [6, 7]]
# nonzero (k_tile, m_tile) pairs of the block-diagonal S^T (3x3 tiling of 384)
BD_KT_FOR_MT = {0: (0, 1), 1: (0, 1, 2), 2: (1, 2)}

_CACHE = {}


def _build_program():
    nc = bacc.Bacc("TRN2", target_bir_lowering=False, debug=False,
                   enable_asserts=False, num_devices=N_CORES)

    x_d = nc.dram_tensor("x", [C, HALF], F32, kind="ExternalInput").ap()
    qkvT_d = nc.dram_tensor("qkvT", [C, 3 * C], F32, kind="ExternalInput").ap()
    projT_d = nc.dram_tensor("projT", [C, C], F32, kind="ExternalInput").ap()
    pb_d = nc.dram_tensor("pb", [C], F32, kind="ExternalInput").ap()
    tau_d = nc.dram_tensor("tau", [96, 4], F32, kind="ExternalInput").ap()
    out_d = nc.dram_tensor("out", [C, HALF], F32, kind="ExternalOutput").ap()

    with tile.TileContext(nc) as tc:
        _emit(nc, tc, x_d, qkvT_d, projT_d, pb_d, tau_d, out_d)

    nc.compile()
    nc.m = get_hw_module(nc.m)
    return nc


def _emit(nc, tc, x_d, qkvT_d, projT_d, pb_d, tau_d, out_d):
    from contextlib import ExitStack
    es = ExitStack()
    # ---- constant / persistent pools -------------------------------------
    const = es.enter_context(tc.tile_pool(name="const", bufs=1))
    wtmp = es.enter_context(tc.tile_pool(name="wtmp", bufs=2))
    vpool = es.enter_context(tc.tile_pool(name="vpool", bufs=1))
    dram = es.enter_context(tc.tile_pool(name="dram", bufs=1, space="DRAM"))

    # weights: DMA raw fp32 then round-cast to fp32r on DVE
    qkvT_sb = []
    for k in range(3):
        raw = wtmp.tile([128, 3 * C], F32, name=f"qkvT_raw{k}", tag="wraw")
        nc.sync.dma_start(raw[:], qkvT_d[128 * k:128 * (k + 1), :])
        t = const.tile([128, 3 * C], F32R, name=f"qkvT{k}")
        nc.vector.tensor_copy(t[:], raw[:])
        qkvT_sb.append(t)
    projT_sb = []
    for k in range(3):
        raw = wtmp.tile([128, C], F32, name=f"projT_raw{k}", tag="wraw2")
        nc.sync.dma_start(raw[:], projT_d[128 * k:128 * (k + 1), :])
        t = const.tile([128, C], F32R, name=f"projT{k}")
        nc.vector.tensor_copy(t[:], raw[:])
        projT_sb.append(t)

    pb_sb = const.tile([128, 3], F32, name="pb_sb")
    nc.sync.dma_start(pb_sb[:], pb_d.rearrange("(f p) -> p f", p=128))
    tau_sb = const.tile([96, 4], F32, name="tau_sb")
    nc.sync.dma_start(tau_sb[:], tau_d[:])

    ones_row = const.tile([1, 128], F32, name="ones_row")
    nc.vector.memset(ones_row[:], 1.0)
    ones_stage = const.tile([128, 1], F32, name="ones_stage")
    nc.vector.memset(ones_stage[:], 1.0)
    ones_col = const.tile([128, 1], F32R, name="ones_col")
    nc.vector.tensor_copy(ones_col[:], ones_stage[:])
    ident = const.tile([128, 128], F32, name="ident")
    make_identity(nc, ident[:])

    # v, persistent across phases: 3 tiles [128, 8192] fp32r
    v_sb = [vpool.tile([128, HALF], F32R, name=f"v_sb{m}") for m in range(3)]

    # collective buffers
    cc_in = dram.tile([98, C], F32, name="cc_in")
    cc_out = dram.tile([98, C], F32, name="cc_out")

    # ---- phase 1: stream chunks, build v / G / sumsq ---------------------
    with tc.tile_pool(name="ph1_psG", bufs=1, space="PSUM") as pG, \
         tc.tile_pool(name="ph1_psS", bufs=1, space="PSUM") as pS, \
         tc.tile_pool(name="ph1_psQK", bufs=2, space="PSUM") as pQK, \
         tc.tile_pool(name="ph1_psV", bufs=3, space="PSUM") as pV, \
         tc.tile_pool(name="ph1_x", bufs=6) as xpool, \
         tc.tile_pool(name="ph1_qk", bufs=4) as qkpool, \
         tc.tile_pool(name="ph1_sq", bufs=4) as sqpool, \
         tc.tile_pool(name="ph1_cc", bufs=1) as ccpool:

        pG_t = pG.tile([96, C], F32, name="pG")
        pS_t = pS.tile([1, 1024], F32, name="pS")   # q sums @ [0:384], k @ [512:896]

        for c in range(NCHUNK):
            cs = CHUNK * c
            xt = []
            for k in range(3):
                t = xpool.tile([128, CHUNK], F32R, name=f"x_{c}_{k}", tag=f"x{k}")
                nc.sync.dma_start(t[:], x_d[128 * k:128 * (k + 1), cs:cs + CHUNK].bitcast(F32R))
                xt.append(t)

            # v = Wv @ x  (3 m-tiles)
            for m in range(3):
                pv = pV.tile([128, CHUNK], F32, name=f"pv_{c}_{m}", tag="v")
                for k in range(3):
                    nc.tensor.matmul(
                        pv[:], qkvT_sb[k][:, 768 + 128 * m:768 + 128 * (m + 1)],
                        xt[k][:], start=(k == 0), stop=(k == 2))
                dst = v_sb[m][:, cs:cs + CHUNK]
                if m == 1:
                    nc.vector.tensor_copy(dst, pv[:])
                else:
                    nc.scalar.copy(dst, pv[:])

            # qT / kT per 128-px subtile, then gram + sumsq accumulation
            for s in range(NSUB_PER_CHUNK):
                st = 128 * s
                first = (c == 0 and s == 0)
                last = (c == NCHUNK - 1 and s == NSUB_PER_CHUNK - 1)
                pq = pQK.tile([128, C], F32, name=f"pq_{c}_{s}", tag="qk")
                pk = pQK.tile([128, C], F32, name=f"pk_{c}_{s}", tag="qk")
                for k in range(3):
                    nc.tensor.matmul(pq[:], xt[k][:, st:st + 128],
                                     qkvT_sb[k][:, 0:C],
                                     start=(k == 0), stop=(k == 2))
                for k in range(3):
                    nc.tensor.matmul(pk[:], xt[k][:, st:st + 128],
                                     qkvT_sb[k][:, C:2 * C],
                                     start=(k == 0), stop=(k == 2))
                qT = qkpool.tile([128, C], F32R, name=f"qT_{c}_{s}", tag="qT")
                kT = qkpool.tile([128, C], F32R, name=f"kT_{c}_{s}", tag="kT")
                nc.scalar.copy(qT[:], pq[:])
                nc.vector.tensor_copy(kT[:], pk[:])

                sqq = sqpool.tile([128, C], F32R, name=f"sqq_{c}_{s}", tag="sqq")
                sqk = sqpool.tile([128, C], F32R, name=f"sqk_{c}_{s}", tag="sqk")
                nc.scalar.square(sqq[:], qT[:])
                nc.vector.tensor_mul(sqk[:], kT[:], kT[:])
                nc.tensor.matmul(pS_t[0:1, 0:C], ones_col[:], sqq[:],
                                 start=first, stop=last)
                nc.tensor.matmul(pS_t[0:1, 512:512 + C], ones_col[:], sqk[:],
                                 start=first, stop=last)
                for p in range(4):
                    nc.tensor.matmul(pG_t[:, 96 * p:96 * (p + 1)],
                                     qT[:, 96 * p:96 * (p + 1)],
                                     kT[:, 96 * p:96 * (p + 1)],
                                     start=first, stop=last)

        # evict G / sumsq to SBUF then DRAM, AllReduce within batch pair
        G_ev = ccpool.tile([96, C], F32, name="G_ev")
        nc.scalar.copy(G_ev[:], pG_t[:])
        sq_ev = ccpool.tile([1, C], F32, name="sq_ev")
        sk_ev = ccpool.tile([1, C], F32, name="sk_ev")
        nc.vector.tensor_copy(sq_ev[:], pS_t[0:1, 0:C])
        nc.vector.tensor_copy(sk_ev[:], pS_t[0:1, 512:512 + C])
        nc.sync.dma_start(cc_in[0:96, :], G_ev[:])
        nc.sync.dma_start(cc_in[96:97, :], sq_ev[:])
        nc.sync.dma_start(cc_in[97:98, :], sk_ev[:])

    nc.gpsimd.collective_compute(
        "AllReduce", ALU.add, replica_groups=GROUPS,
        ins=[cc_in.opt()], outs=[cc_out.opt()])

    # ---- phase 2/3: norms, softmax, block-diagonal S^T -------------------
    st_bd = [vpool.tile([128, C], F32R, name=f"st_bd{t}") for t in range(3)]
    zstage = vpool.tile([128, C], F32, name="zstage")
    nc.vector.memset(zstage[:], 0.0)
    for t in range(3):
        nc.vector.tensor_copy(st_bd[t][:], zstage[:])

    with tc.tile_pool(name="ph2_ps", bufs=2, space="PSUM") as pT, \
         tc.tile_pool(name="ph2_sb", bufs=1) as sm:

        # G rows are re-spread so each head's 48 d-rows start at a 64-aligned
        # partition (DVE/ACT require 32-aligned partition bases).
        G_sb = sm.tile([128, C], F32, name="G_sb")
        nc.sync.dma_start(G_sb[0:48, :], cc_out[0:48, :])
        nc.sync.dma_start(G_sb[64:112, :], cc_out[48:96, :])
        sq_cols = sm.tile([96, 8], F32, name="sq_cols")
        nc.sync.dma_start(sq_cols[:, 0:4],
                          cc_out[96:97, :].rearrange("a (f p) -> p (a f)", p=96))
        nc.sync.dma_start(sq_cols[:, 4:8],
                          cc_out[97:98, :].rearrange("a (f p) -> p (a f)", p=96))

        # inv = min(sqrt(1/s), 1e12)  ( == 1/max(sqrt(s), 1e-12) for s >= 0 )
        inv_cols = sm.tile([96, 8], F32, name="inv_cols")
        nc.vector.reciprocal(inv_cols[:], sq_cols[:])
        nc.scalar.sqrt(inv_cols[:], inv_cols[:])
        nc.vector.tensor_scalar_min(inv_cols[:], inv_cols[:], 1e12)
        a_cols = sm.tile([96, 4], F32, name="a_cols")
        nc.vector.tensor_mul(a_cols[:], inv_cols[:, 0:4], tau_sb[:])
        a_pad = sm.tile([128, 4], F32, name="a_pad")
        nc.sync.dma_start(a_pad[0:48, :], a_cols[0:48, :])
        nc.sync.dma_start(a_pad[64:112, :], a_cols[48:96, :])

        # invk rows: transpose each k-column [96,1] -> [1,96] at partition 0
        b_rows = []
        for p in range(4):
            ptr = pT.tile([1, 96], F32, name=f"ptr{p}", tag="tr1")
            nc.tensor.transpose(ptr[:], inv_cols[:, 4 + p:5 + p], ident[0:96, 0:96])
            br = sm.tile([1, 96], F32, name=f"b_row{p}")
            nc.scalar.copy(br[:], ptr[:])
            b_rows.append(br)
        # broadcast each row to a [128, 96] matrix via ones ⊗ row (rows all
        # identical, so the 64-aligned padded row layout is free)
        B_sb = []
        for p in range(4):
            pB = pT.tile([128, 96], F32, name=f"pB{p}", tag="bc")
            nc.tensor.matmul(pB[:], ones_row[:], b_rows[p][:], start=True, stop=True)
            Bs = sm.tile([128, 96], F32, name=f"B_sb{p}")
            nc.vector.tensor_copy(Bs[:], pB[:])
            B_sb.append(Bs)

        # softmax per head on 48x48 blocks (free-dim ops only); head d-rows
        # live at 64-aligned partition bases (0 / 64)
        S_sb = sm.tile([128, C], F32, name="S_sb")
        nc.vector.memset(S_sb[:], 0.0)
        Lt = sm.tile([128, HD], F32, name="Lt")
        negm = sm.tile([128, 1], F32, name="negm")
        den = sm.tile([128, 1], F32, name="den")
        rden = sm.tile([128, 1], F32, name="rden")
        for n in range(NH):
            p, j = n // 2, n % 2
            rs = slice(64 * j, 64 * j + HD)
            cs_ = slice(96 * p + HD * j, 96 * p + HD * (j + 1))
            bs = slice(HD * j, HD * (j + 1))
            G_blk = G_sb[rs, cs_]
            L = Lt[rs, :]
            nc.vector.scalar_tensor_tensor(
                L, G_blk, a_pad[rs, p:p + 1], B_sb[p][rs, bs],
                op0=ALU.mult, op1=ALU.mult)
            nc.vector.reduce_max(negm[rs, :], L, axis=AX.X, negate=True)
            nc.scalar.activation(S_sb[rs, cs_], L, AFT.Exp,
                                 bias=negm[rs, :], scale=1.0,
                                 accum_out=den[rs, :])
            nc.vector.reciprocal(rden[rs, :], den[rs, :])
            nc.vector.tensor_scalar_mul(S_sb[rs, cs_], S_sb[rs, cs_], rden[rs, :])

        # transpose each pair block [128, 96] -> [96, 128], scatter the two
        # real 48-col pieces (d-cols 0:48 and 64:112) into block-diag S^T
        for p in range(4):
            pSp = pT.tile([96, 128], F32, name=f"pS{p}", tag="tr2")
            nc.tensor.transpose(pSp[:], S_sb[:, 96 * p:96 * (p + 1)],
                                ident[:, :])
            stg = sm.tile([96, 128], F32, name=f"stg{p}", tag="stg")
            nc.scalar.copy(stg[:], pSp[:])
            for src_c, dst_c in ((0, 0), (64, HD)):
                r0 = 96 * p
                while r0 < 96 * (p + 1):
                    t_i = r0 // 128
                    take = min(128 * (t_i + 1) - r0, 96 * (p + 1) - r0)
                    src0 = r0 - 96 * p
                    nc.sync.dma_start(
                        st_bd[t_i][r0 - 128 * t_i:r0 - 128 * t_i + take,
                                   96 * p + dst_c:96 * p + dst_c + HD],
                        stg[src0:src0 + take, src_c:src_c + HD].bitcast(F32R))
                    r0 += take

    # ---- phase 4: out = S_bd^T @ v ; y = projT^T @ out + b ---------------
    with tc.tile_pool(name="ph4_psO", bufs=4, space="PSUM") as pO, \
         tc.tile_pool(name="ph4_psP", bufs=4, space="PSUM") as pP, \
         tc.tile_pool(name="ph4_o", bufs=6) as opool, \
         tc.tile_pool(name="ph4_f", bufs=6) as fpool:

        for c in range(NCHUNK):
            cs = CHUNK * c
            osb = []
            for mt in range(3):
                po = pO.tile([128, CHUNK], F32, name=f"po_{c}_{mt}", tag="o")
                kts = BD_KT_FOR_MT[mt]
                for i, kt in enumerate(kts):
                    nc.tensor.matmul(po[:],
                                     st_bd[kt][:, 128 * mt:128 * (mt + 1)],
                                     v_sb[kt][:, cs:cs + CHUNK],
                                     start=(i == 0), stop=(i == len(kts) - 1))
                ot = opool.tile([128, CHUNK], F32R, name=f"ot_{c}_{mt}", tag="ot")
                if mt == 1:
                    nc.vector.tensor_copy(ot[:], po[:])
                else:
                    nc.scalar.copy(ot[:], po[:])
                osb.append(ot)
            for mt in range(3):
                pp = pP.tile([128, CHUNK], F32, name=f"pp_{c}_{mt}", tag="p")
                for kt in range(3):
                    nc.tensor.matmul(pp[:],
                                     projT_sb[kt][:, 128 * mt:128 * (mt + 1)],
                                     osb[kt][:], start=(kt == 0), stop=(kt == 2))
                ft = fpool.tile([128, CHUNK], F32, name=f"ft_{c}_{mt}", tag="ft")
                nc.scalar.activation(ft[:], pp[:], AFT.Identity,
                                     bias=pb_sb[:, mt:mt + 1], scale=1.0)
                nc.sync.dma_start(out_d[128 * mt:128 * (mt + 1), cs:cs + CHUNK],
                                  ft[:])

    es.close()


def kernel(x, qkv_w, proj_w, proj_b, temperature):
    x = np.asarray(x, dtype=np.float32)
    qkv_w = np.asarray(qkv_w, dtype=np.float32)
    proj_w = np.asarray(proj_w, dtype=np.float32)
    proj_b = np.asarray(proj_b, dtype=np.float32)
    temperature = np.asarray(temperature, dtype=np.float32).reshape(NH)

    B = x.shape[0]
    qkvT = np.ascontiguousarray(qkv_w.T)
    projT = np.ascontiguousarray(proj_w.T)
    tau_cols = np.empty((96, 4), np.float32)
    for p in range(4):
        tau_cols[0:HD, p] = temperature[2 * p]
        tau_cols[HD:96, p] = temperature[2 * p + 1]

    if "prog" not in _CACHE:
        _CACHE["prog"] = _build_program()
    nc = _CACHE["prog"]

    in_maps = []
    for i in range(N_CORES):
        b, h = i // 2, i % 2
        xi = np.ascontiguousarray(x[b].reshape(C, HW)[:, h * HALF:(h + 1) * HALF])
        in_maps.append({"x": xi, "qkvT": qkvT, "projT": projT,
                        "pb": proj_b, "tau": tau_cols})

    res = run_bass_kernel_spmd(nc, in_maps, core_ids=list(range(N_CORES)))

    out = np.empty((B, C, 128, 128), np.float32)
    for i in range(N_CORES):
        b, h = i // 2, i % 2
        out[b].reshape(C, HW)[:, h * HALF:(h + 1) * HALF] = res.results[i]["out"]
    return out


# revision 15
# speedup vs baseline: 1.0535x; 1.0535x over previous
"""Trainium2 Bass kernel for nn_ContextAggregation (channel attention).

Reference computation (per batch b of 4, c=384, nh=8, hd=48, hw=128*128):
    qkv  = qkv_w @ x            # [1152, hw] channel GEMM
    q,k,v split; q,k L2-normalized along hw
    attn = softmax((q_hat @ k_hat^T) * tau)   # [nh, 48, 48]
    out  = proj_w @ (attn @ v) + proj_b

Sharding: 8 cores = (batch b, hw-half h).  Each core processes hw half
(8192 px) of one batch.  The only cross-core data needed are the gram
matrices q@k^T and the row sums-of-squares (for the L2 norms), both of
which are sums over hw -> one tiny pair AllReduce ([98,384] floats)
between the two cores sharing a batch.  All heavy GEMMs are local.

Per-core pipeline (single Bass program, SPMD on cores 0-7):
  phase 1 (stream 16 chunks of 512 px):
    - v    = Wv @ x                        (natural [ch, hw] layout, fp32r)
    - qT,kT= x^T @ Wq^T / Wk^T             (transposed [hw, ch] layout: the
             x subtile is the stationary operand, so q/k come out hw-major
             and the gram contraction over hw can use the PE directly)
    - G   += qT_pair^T @ kT_pair           (4 head-pairs, PSUM-accumulated)
    - ssq += ones^T @ qT^2, ssk            (PSUM-accumulated sumsq)
  AllReduce(G|ssq|ssk) within batch pairs.
  phase 2/3: inverse norms, softmax per head, transpose S via PE,
             scatter into a block-diagonal S^T (384x384, fp32r).
  phase 4 (stream 16 chunks):
    - out  = S_bd^T @ v                    (skips all-zero block tiles)
    - y    = projT^T @ out + b             -> DMA to DRAM
"""

import numpy as np

import concourse.bass as bass
import concourse.mybir as mybir
import concourse.tile as tile
from concourse import bacc
from concourse.bass_utils import run_bass_kernel_spmd
from concourse.bass_interp import get_hw_module
from concourse.masks import make_identity

dt = mybir.dt
F32 = dt.float32
F32R = dt.float32r
ALU = mybir.AluOpType
AFT = mybir.ActivationFunctionType
AX = mybir.AxisListType

C = 384          # channels
NH = 8           # heads
HD = 48          # head dim
HW = 128 * 128   # pixels per image
HALF = HW // 2   # pixels per core
CHUNK = 512
NCHUNK = HALF // CHUNK          # 16
SUB = 128
NSUB_PER_CHUNK = CHUNK // SUB   # 4
N_CORES = 8
GROUPS = [[0, 1], [2, 3], [4, 5], [6, 7]]
# nonzero (k_tile, m_tile) pairs of the block-diagonal S^T (3x3 tiling of 384)
BD_KT_FOR_MT = {0: (0, 1), 1: (0, 1, 2), 2: (1, 2)}

_CACHE = {}


def _build_program():
    nc = bacc.Bacc("TRN2", target_bir_lowering=False, debug=False,
                   enable_asserts=False, num_devices=N_CORES)

    x_d = nc.dram_tensor("x", [C, HALF], F32, kind="ExternalInput").ap()
    qkvT_d = nc.dram_tensor("qkvT", [C, 3 * C], F32, kind="ExternalInput").ap()
    projT_d = nc.dram_tensor("projT", [C, C], F32, kind="ExternalInput").ap()
    pb_d = nc.dram_tensor("pb", [C], F32, kind="ExternalInput").ap()
    tau_d = nc.dram_tensor("tau", [96, 4], F32, kind="ExternalInput").ap()
    out_d = nc.dram_tensor("out", [C, HALF], F32, kind="ExternalOutput").ap()

    with tile.TileContext(nc) as tc:
        _emit(nc, tc, x_d, qkvT_d, projT_d, pb_d, tau_d, out_d)

    nc.compile()
    nc.m = get_hw_module(nc.m)
    return nc


def _emit(nc, tc, x_d, qkvT_d, projT_d, pb_d, tau_d, out_d):
    from contextlib import ExitStack
    es = ExitStack()
    # ---- constant / persistent pools -------------------------------------
    const = es.enter_context(tc.tile_pool(name="const", bufs=1))
    vpool = es.enter_context(tc.tile_pool(name="vpool", bufs=1))
    dram = es.enter_context(tc.tile_pool(name="dram", bufs=1, space="DRAM"))
    xpool = es.enter_context(tc.tile_pool(name="ph1_x", bufs=4))

    def x_tiles(c, phase):
        cs = CHUNK * c
        ts = []
        for k in range(3):
            t = xpool.tile([128, CHUNK], F32R, name=f"x{phase}_{c}_{k}",
                           tag=f"x{k}")
            nc.sync.dma_start(
                t[:], x_d[128 * k:128 * (k + 1), cs:cs + CHUNK].bitcast(F32R))
            ts.append(t)
        return ts

    # prefetch chunk 0 before anything else so the PE can start ASAP
    xt0 = x_tiles(0, "a")

    # weights: DMA straight into fp32r tiles (raw bits; PE rounds on ingest)
    qkvT_sb = []
    for k in range(3):
        t = const.tile([128, 3 * C], F32R, name=f"qkvT{k}")
        nc.sync.dma_start(t[:, 0:2 * C],
                          qkvT_d[128 * k:128 * (k + 1), 0:2 * C].bitcast(F32R))
        qkvT_sb.append(t)
    ones_stage = const.tile([128, 1], F32, name="ones_stage")
    nc.vector.memset(ones_stage[:], 1.0)
    ones_col = const.tile([128, 1], F32R, name="ones_col")
    nc.vector.tensor_copy(ones_col[:], ones_stage[:])

    # v, persistent across phases: 3 tiles [128, 8192] fp32r
    v_sb = [vpool.tile([128, HALF], F32R, name=f"v_sb{m}") for m in range(3)]

    # collective buffers
    cc_in = dram.tile([98, C], F32, name="cc_in")
    cc_out = dram.tile([98, C], F32, name="cc_out")

    # ---- phase 1a: stream chunks, build G / sumsq (q,k only) -------------
    with tc.tile_pool(name="ph1_psG", bufs=1, space="PSUM") as pG, \
         tc.tile_pool(name="ph1_psS", bufs=1, space="PSUM") as pS, \
         tc.tile_pool(name="ph1_psQK", bufs=4, space="PSUM") as pQK, \
         tc.tile_pool(name="ph1_qk", bufs=4) as qkpool, \
         tc.tile_pool(name="ph1_sq", bufs=4) as sqpool, \
         tc.tile_pool(name="ph1_cc", bufs=1) as ccpool:

        pG_t = pG.tile([96, C], F32, name="pG")
        pS_t = pS.tile([1, 1024], F32, name="pS")   # q sums @ [0:384], k @ [512:896]

        for c in range(NCHUNK):
            xt = xt0 if c == 0 else x_tiles(c, "a")
            # qT / kT per 128-px subtile, then gram + sumsq accumulation
            for s in range(NSUB_PER_CHUNK):
                st = 128 * s
                first = (c == 0 and s == 0)
                last = (c == NCHUNK - 1 and s == NSUB_PER_CHUNK - 1)
                pq = pQK.tile([128, C], F32, name=f"pq_{c}_{s}", tag="qk")
                pk = pQK.tile([128, C], F32, name=f"pk_{c}_{s}", tag="qk")
                for k in range(3):
                    nc.tensor.matmul(pq[:], xt[k][:, st:st + 128],
                                     qkvT_sb[k][:, 0:C],
                                     start=(k == 0), stop=(k == 2))
                for k in range(3):
                    nc.tensor.matmul(pk[:], xt[k][:, st:st + 128],
                                     qkvT_sb[k][:, C:2 * C],
                                     start=(k == 0), stop=(k == 2))
                qT = qkpool.tile([128, C], F32R, name=f"qT_{c}_{s}", tag="qT")
                kT = qkpool.tile([128, C], F32R, name=f"kT_{c}_{s}", tag="kT")
                nc.scalar.copy(qT[:], pq[:])
                nc.vector.tensor_copy(kT[:], pk[:])

                sqq = sqpool.tile([128, C], F32R, name=f"sqq_{c}_{s}", tag="sqq")
                sqk = sqpool.tile([128, C], F32R, name=f"sqk_{c}_{s}", tag="sqk")
                nc.scalar.square(sqq[:], qT[:])
                nc.vector.tensor_mul(sqk[:], kT[:], kT[:])
                nc.tensor.matmul(pS_t[0:1, 0:C], ones_col[:], sqq[:],
                                 start=first, stop=last)
                nc.tensor.matmul(pS_t[0:1, 512:512 + C], ones_col[:], sqk[:],
                                 start=first, stop=last)
                for p in range(4):
                    nc.tensor.matmul(pG_t[:, 96 * p:96 * (p + 1)],
                                     qT[:, 96 * p:96 * (p + 1)],
                                     kT[:, 96 * p:96 * (p + 1)],
                                     start=first, stop=last)

        # evict G / sumsq to SBUF then DRAM, AllReduce within batch pair
        G_ev = ccpool.tile([96, C], F32, name="G_ev")
        nc.scalar.copy(G_ev[:], pG_t[:])
        sq_ev = ccpool.tile([1, C], F32, name="sq_ev")
        sk_ev = ccpool.tile([1, C], F32, name="sk_ev")
        nc.vector.tensor_copy(sq_ev[:], pS_t[0:1, 0:C])
        nc.vector.tensor_copy(sk_ev[:], pS_t[0:1, 512:512 + C])
        nc.sync.dma_start(cc_in[0:96, :], G_ev[:])
        nc.sync.dma_start(cc_in[96:97, :], sq_ev[:])
        nc.sync.dma_start(cc_in[97:98, :], sk_ev[:])

    nc.gpsimd.collective_compute(
        "AllReduce", ALU.add, replica_groups=GROUPS,
        ins=[cc_in.opt()], outs=[cc_out.opt()])

    # ---- phase 1b: v = Wv @ x (overlaps the AllReduce + softmax) ---------
    nc.sync.dma_start(qkvT_sb[0][:, 2 * C:3 * C],
                      qkvT_d[0:128, 2 * C:3 * C].bitcast(F32R))
    nc.sync.dma_start(qkvT_sb[1][:, 2 * C:3 * C],
                      qkvT_d[128:256, 2 * C:3 * C].bitcast(F32R))
    nc.sync.dma_start(qkvT_sb[2][:, 2 * C:3 * C],
                      qkvT_d[256:384, 2 * C:3 * C].bitcast(F32R))
    with tc.tile_pool(name="ph1b_psV", bufs=6, space="PSUM") as pV:
        for c in range(NCHUNK):
            cs = CHUNK * c
            xt = x_tiles(c, "b")
            for m in range(3):
                pv = pV.tile([128, CHUNK], F32, name=f"pv_{c}_{m}", tag="v")
                for k in range(3):
                    nc.tensor.matmul(
                        pv[:], qkvT_sb[k][:, 768 + 128 * m:768 + 128 * (m + 1)],
                        xt[k][:], start=(k == 0), stop=(k == 2))
                dst = v_sb[m][:, cs:cs + CHUNK]
                if m == 1:
                    nc.vector.tensor_copy(dst, pv[:])
                else:
                    nc.scalar.copy(dst, pv[:])

    # ---- phase 2/3: norms, softmax, block-diagonal S^T -------------------
    projT_sb = []
    for k in range(3):
        t = const.tile([128, C], F32R, name=f"projT{k}")
        nc.sync.dma_start(t[:], projT_d[128 * k:128 * (k + 1), :].bitcast(F32R))
        projT_sb.append(t)
    pb_sb = const.tile([128, 3], F32, name="pb_sb")
    nc.sync.dma_start(pb_sb[:], pb_d.rearrange("(f p) -> p f", p=128))
    tau_sb = const.tile([96, 4], F32, name="tau_sb")
    nc.sync.dma_start(tau_sb[:], tau_d[:])
    ones_row = const.tile([1, 128], F32, name="ones_row")
    nc.vector.memset(ones_row[:], 1.0)
    ident = const.tile([128, 128], F32, name="ident")
    make_identity(nc, ident[:])

    st_bd = [vpool.tile([128, C], F32R, name=f"st_bd{t}") for t in range(3)]
    zstage = vpool.tile([128, C], F32, name="zstage")
    nc.vector.memset(zstage[:], 0.0)
    for t in range(3):
        nc.vector.tensor_copy(st_bd[t][:], zstage[:])

    with tc.tile_pool(name="ph2_ps", bufs=2, space="PSUM") as pT, \
         tc.tile_pool(name="ph2_sb", bufs=1) as sm:

        # G rows are re-spread so each head's 48 d-rows start at a 64-aligned
        # partition (DVE/ACT require 32-aligned partition bases).
        G_sb = sm.tile([128, C], F32, name="G_sb")
        nc.sync.dma_start(G_sb[0:48, :], cc_out[0:48, :])
        nc.sync.dma_start(G_sb[64:112, :], cc_out[48:96, :])
        sq_cols = sm.tile([96, 8], F32, name="sq_cols")
        nc.sync.dma_start(sq_cols[:, 0:4],
                          cc_out[96:97, :].rearrange("a (f p) -> p (a f)", p=96))
        nc.sync.dma_start(sq_cols[:, 4:8],
                          cc_out[97:98, :].rearrange("a (f p) -> p (a f)", p=96))

        # inv = min(sqrt(1/s), 1e12)  ( == 1/max(sqrt(s), 1e-12) for s >= 0 )
        inv_cols = sm.tile([96, 8], F32, name="inv_cols")
        nc.vector.reciprocal(inv_cols[:], sq_cols[:])
        nc.scalar.sqrt(inv_cols[:], inv_cols[:])
        nc.vector.tensor_scalar_min(inv_cols[:], inv_cols[:], 1e12)
        a_cols = sm.tile([96, 4], F32, name="a_cols")
        nc.vector.tensor_mul(a_cols[:], inv_cols[:, 0:4], tau_sb[:])
        a_pad = sm.tile([128, 4], F32, name="a_pad")
        nc.sync.dma_start(a_pad[0:48, :], a_cols[0:48, :])
        nc.sync.dma_start(a_pad[64:112, :], a_cols[48:96, :])

        # invk rows: transpose each k-column [96,1] -> [1,96] at partition 0
        b_rows = []
        for p in range(4):
            ptr = pT.tile([1, 96], F32, name=f"ptr{p}", tag="tr1")
            nc.tensor.transpose(ptr[:], inv_cols[:, 4 + p:5 + p], ident[0:96, 0:96])
            br = sm.tile([1, 96], F32, name=f"b_row{p}")
            nc.scalar.copy(br[:], ptr[:])
            b_rows.append(br)
        # broadcast each row to a [128, 96] matrix via ones ⊗ row (rows all
        # identical, so the 64-aligned padded row layout is free)
        B_sb = []
        for p in range(4):
            pB = pT.tile([128, 96], F32, name=f"pB{p}", tag="bc")
            nc.tensor.matmul(pB[:], ones_row[:], b_rows[p][:], start=True, stop=True)
            Bs = sm.tile([128, 96], F32, name=f"B_sb{p}")
            nc.vector.tensor_copy(Bs[:], pB[:])
            B_sb.append(Bs)

        # softmax per head on 48x48 blocks (free-dim ops only); head d-rows
        # live at 64-aligned partition bases (0 / 64)
        S_sb = sm.tile([128, C], F32, name="S_sb")
        nc.vector.memset(S_sb[:], 0.0)
        Lt = sm.tile([128, HD], F32, name="Lt")
        negm = sm.tile([128, 1], F32, name="negm")
        den = sm.tile([128, 1], F32, name="den")
        rden = sm.tile([128, 1], F32, name="rden")
        for n in range(NH):
            p, j = n // 2, n % 2
            rs = slice(64 * j, 64 * j + HD)
            cs_ = slice(96 * p + HD * j, 96 * p + HD * (j + 1))
            bs = slice(HD * j, HD * (j + 1))
            G_blk = G_sb[rs, cs_]
            L = Lt[rs, :]
            nc.vector.scalar_tensor_tensor(
                L, G_blk, a_pad[rs, p:p + 1], B_sb[p][rs, bs],
                op0=ALU.mult, op1=ALU.mult)
            nc.vector.reduce_max(negm[rs, :], L, axis=AX.X, negate=True)
            nc.scalar.activation(S_sb[rs, cs_], L, AFT.Exp,
                                 bias=negm[rs, :], scale=1.0,
                                 accum_out=den[rs, :])
            nc.vector.reciprocal(rden[rs, :], den[rs, :])
            nc.vector.tensor_scalar_mul(S_sb[rs, cs_], S_sb[rs, cs_], rden[rs, :])

        # transpose each pair block [128, 96] -> [96, 128], scatter the two
        # real 48-col pieces (d-cols 0:48 and 64:112) into block-diag S^T
        for p in range(4):
            pSp = pT.tile([96, 128], F32, name=f"pS{p}", tag="tr2")
            nc.tensor.transpose(pSp[:], S_sb[:, 96 * p:96 * (p + 1)],
                                ident[:, :])
            stg = sm.tile([96, 128], F32, name=f"stg{p}", tag="stg")
            nc.scalar.copy(stg[:], pSp[:])
            for src_c, dst_c in ((0, 0), (64, HD)):
                r0 = 96 * p
                while r0 < 96 * (p + 1):
                    t_i = r0 // 128
                    take = min(128 * (t_i + 1) - r0, 96 * (p + 1) - r0)
                    src0 = r0 - 96 * p
                    nc.sync.dma_start(
                        st_bd[t_i][r0 - 128 * t_i:r0 - 128 * t_i + take,
                                   96 * p + dst_c:96 * p + dst_c + HD],
                        stg[src0:src0 + take, src_c:src_c + HD].bitcast(F32R))
                    r0 += take

    # ---- phase 4: out = S_bd^T @ v ; y = projT^T @ out + b ---------------
    with tc.tile_pool(name="ph4_psO", bufs=4, space="PSUM") as pO, \
         tc.tile_pool(name="ph4_psP", bufs=4, space="PSUM") as pP, \
         tc.tile_pool(name="ph4_o", bufs=4) as opool, \
         tc.tile_pool(name="ph4_f", bufs=4) as fpool:

        for c in range(NCHUNK):
            cs = CHUNK * c
            osb = []
            for mt in range(3):
                po = pO.tile([128, CHUNK], F32, name=f"po_{c}_{mt}", tag="o")
                kts = BD_KT_FOR_MT[mt]
                for i, kt in enumerate(kts):
                    nc.tensor.matmul(po[:],
                                     st_bd[kt][:, 128 * mt:128 * (mt + 1)],
                                     v_sb[kt][:, cs:cs + CHUNK],
                                     start=(i == 0), stop=(i == len(kts) - 1))
                ot = opool.tile([128, CHUNK], F32R, name=f"ot_{c}_{mt}", tag="ot")
                if mt == 1:
                    nc.vector.tensor_copy(ot[:], po[:])
                else:
                    nc.scalar.copy(ot[:], po[:])
                osb.append(ot)
            for mt in range(3):
                pp = pP.tile([128, CHUNK], F32, name=f"pp_{c}_{mt}", tag="p")
                for kt in range(3):
                    nc.tensor.matmul(pp[:],
                                     projT_sb[kt][:, 128 * mt:128 * (mt + 1)],
                                     osb[kt][:], start=(kt == 0), stop=(kt == 2))
                ft = fpool.tile([128, CHUNK], F32, name=f"ft_{c}_{mt}", tag="ft")
                nc.scalar.activation(ft[:], pp[:], AFT.Identity,
                                     bias=pb_sb[:, mt:mt + 1], scale=1.0)
                nc.sync.dma_start(out_d[128 * mt:128 * (mt + 1), cs:cs + CHUNK],
                                  ft[:])

    es.close()


def kernel(x, qkv_w, proj_w, proj_b, temperature):
    x = np.asarray(x, dtype=np.float32)
    qkv_w = np.asarray(qkv_w, dtype=np.float32)
    proj_w = np.asarray(proj_w, dtype=np.float32)
    proj_b = np.asarray(proj_b, dtype=np.float32)
    temperature = np.asarray(temperature, dtype=np.float32).reshape(NH)

    B = x.shape[0]
    qkvT = np.ascontiguousarray(qkv_w.T)
    projT = np.ascontiguousarray(proj_w.T)
    tau_cols = np.empty((96, 4), np.float32)
    for p in range(4):
        tau_cols[0:HD, p] = temperature[2 * p]
        tau_cols[HD:96, p] = temperature[2 * p + 1]

    if "prog" not in _CACHE:
        _CACHE["prog"] = _build_program()
    nc = _CACHE["prog"]

    in_maps = []
    for i in range(N_CORES):
        b, h = i // 2, i % 2
        xi = np.ascontiguousarray(x[b].reshape(C, HW)[:, h * HALF:(h + 1) * HALF])
        in_maps.append({"x": xi, "qkvT": qkvT, "projT": projT,
                        "pb": proj_b, "tau": tau_cols})

    res = run_bass_kernel_spmd(nc, in_maps, core_ids=list(range(N_CORES)))

    out = np.empty((B, C, 128, 128), np.float32)
    for i in range(N_CORES):
        b, h = i // 2, i % 2
        out[b].reshape(C, HW)[:, h * HALF:(h + 1) * HALF] = res.results[i]["out"]
    return out
